# revision 4
# baseline (speedup 1.0000x reference)
"""ComplexMultiHeadAttention on 8 TRN2 NeuronCores (Bass/Tile).

Problem: B=4, S=1024, D_MODEL=1024, N_HEADS=16, D_HEAD=64, complex-valued
activations stored as a trailing dim of size 2 (real, imag).

    q = to_heads(complex_linear(queries, wq));  k, v likewise
    s_r + i*s_i = (q_r + i q_i)(k_r + i k_i)^T / sqrt(dh)
    a_r = softmax(s_r), a_i = softmax(s_i)      (independent softmaxes)
    o = complex_bmm(a, v);  out = complex_linear(concat_heads(o), wo)

Sharding: head-parallel. Core c owns heads {2c, 2c+1} = 128 contiguous dims
of the hidden axis. Each core computes Q/K/V projections for its 128 output
dims (weights row-sliced), runs attention for its 8 (batch, head) pairs, and
computes a partial O-projection (wo column-sliced on its 128 input dims)
over all 1024 output dims. The host sums the 8 partial outputs — no
on-device collectives.

Layout: tokens always on the FREE dim, features/keys on partitions, so
every matmul is a natural lhsT.T @ rhs with K=128 contraction.

Q/K/V projections use Gauss's 3-multiplication complex product: per head
   T1 = wr@(xr+xi), T2 = (wr+wi)@xi, T3 = (wi-wr)@xr
   out_r = T1-T2, out_i = T1+T3
with (xr+xi) precomputed on the HOST and uploaded as a third bf16 input
stream (trades DMA bytes, which have slack, for a 25% cut in projection
matmuls). The T1/T2/T3 psums pack both heads in M=128; the combines run
on DVE while reading T2/T3 straight from PSUM (T1 staged to SBUF once).

Scores are computed TRANSPOSED (s^T [k, q]) from Qcat=[q_r;q_i],
Kcat_r=[k_r;-k_i], Kcat_i=[k_i;k_r] (f32r) — one K=128 matmul per 128-key
chunk. Softmax over k (=partitions) skips max-subtraction (scores are O(1)
by construction). exp writes bf16 u-tiles; the 8 per-part u tiles are
pairwise tree-summed on DVE and ONE ones[128,128]-matmul replicates the
total Z across partitions, making the 1/Z scale an aligned tensor_mul.
Z psums allocate from the score-psum ring (sps) so the two Z matmuls of a
group land in different banks — no recip->matmul serialization.

V is PE-transposed into token-major packs VA=[v_r|v_i], VB=[v_i|v_r], so
attn@V accumulates o_pack [o_r|o_i, q] in a single psum group.

Scheduling: the PE p-state drops (~2x slower matmuls) whenever the engine
idles, so projection work for batch b+1 and the O-projections of batch b
are interleaved between attention groups of batch b to keep the PE queue
non-empty across group-end dependency bubbles. Weights load in first-use
order so the first matmul starts ~2us in, not after all weight DMA.

Matmul dtype note (cost-model + HW verified): bf16 and f32r both run at
1 cycle/row for >=256-row moving operands, so dtype choice is about DMA
bytes and precision, not PE speed. Scores/Q/K stay f32r in SBUF (exp is
the error-sensitive consumer); x/weights/u/V/outputs are bf16.
"""

import os
import numpy as np
import ml_dtypes
from contextlib import ExitStack

import concourse.bass as bass
import concourse.tile as tile
from concourse import bacc, mybir
from concourse.alu_op_type import AluOpType

F32 = mybir.dt.float32
F32R = mybir.dt.float32r
BF16 = mybir.dt.bfloat16
EXP = mybir.ActivationFunctionType.Exp

B, S, D, H, DH = 4, 1024, 1024, 16, 64
NCORES = 8
P = 128            # partitions / chunk size
TBLK = 512         # token block (matmul free dim)
DC = D // P        # 8 d-chunks
KC = S // P        # 8 key chunks per batch
HPC = H // NCORES  # 2 heads per core
NT = (B * S) // TBLK  # 8 token blocks

_CACHE = {}


def _build():
    nc = bacc.Bacc("TRN2", target_bir_lowering=False, debug=False,
                   num_devices=NCORES)

    x_ap = {}
    for t in ("q", "k", "v"):
        for part in ("r", "i", "s"):   # s = host-precomputed (xr + xi)
            # tiled-contiguous layout: row block (dc*NT + gt)*P : +P is one
            # [128, 512] tile stored contiguously (single-descriptor DMA)
            x_ap[t + part] = nc.dram_tensor(
                f"x{t}_{part}", [DC * NT * P, TBLK],
                BF16, kind="ExternalInput").ap()
    # Gauss weight packs, both heads in M: w1=[wr_h0|wr_h1],
    # w2=[(wr+wi)_h0|(wr+wi)_h1], w3=[(wi-wr)_h0|(wi-wr)_h1]
    w_ap = {}
    for t in ("q", "k", "v"):
        for j in (1, 2, 3):
            w_ap[f"{t}{j}"] = nc.dram_tensor(
                f"w{t}_{j}", [P, D], BF16, kind="ExternalInput").ap()
    wo_ap = {}
    for suf in ("r", "i", "in"):
        wo_ap[suf] = nc.dram_tensor(
            f"wo_{suf}", [P, D], BF16, kind="ExternalInput").ap()
    ident_ap = nc.dram_tensor("ident", [P, P], BF16, kind="ExternalInput").ap()
    ones_ap = nc.dram_tensor("onesin", [P, P], BF16, kind="ExternalInput").ap()
    # same tiled-contiguous trick for outputs: row block (gt*DC + mc)*P
    po_r = nc.dram_tensor("po_r", [NT * DC * P, TBLK], BF16,
                          kind="ExternalOutput").ap()
    po_i = nc.dram_tensor("po_i", [NT * DC * P, TBLK], BF16,
                          kind="ExternalOutput").ap()

    with tile.TileContext(nc) as tc, ExitStack() as ctx:
        wpool = ctx.enter_context(tc.tile_pool(name="w", bufs=1))
        xpool = ctx.enter_context(tc.tile_pool(name="x", bufs=12))
        qkpool = ctx.enter_context(tc.tile_pool(name="qk", bufs=2))
        vpool = ctx.enter_context(tc.tile_pool(name="v", bufs=2))
        opool = ctx.enter_context(tc.tile_pool(name="ost", bufs=2))
        upool = ctx.enter_context(tc.tile_pool(name="u", bufs=8))
        uspool = ctx.enter_context(tc.tile_pool(name="us", bufs=8))
        t1pool = ctx.enter_context(tc.tile_pool(name="t1", bufs=2))
        zpool = ctx.enter_context(tc.tile_pool(name="z", bufs=2))
        tmppool = ctx.enter_context(tc.tile_pool(name="tmp", bufs=4))
        popool = ctx.enter_context(tc.tile_pool(name="po", bufs=4))
        vstpool = ctx.enter_context(tc.tile_pool(name="vst", bufs=2))
        # PSUM: 8 banks. projps (3) holds the Gauss T1/T2/T3 accumulators;
        # sps (3) rotates score tiles, Z sums, the V-transpose target and
        # the O-projection accumulators; ops 2 (ota+otb).
        projps = ctx.enter_context(tc.tile_pool(name="pp", bufs=3, space="PSUM"))
        sps = ctx.enter_context(tc.tile_pool(name="sp", bufs=3, space="PSUM"))
        ops_pool = ctx.enter_context(tc.tile_pool(name="op", bufs=1, space="PSUM"))

        wt = {}

        def load_w(key):
            wt[key] = wpool.tile([P, D], BF16, tag=f"w_{key}", name=f"w_{key}")
            nc.sync.dma_start(wt[key][:], w_ap[key][:])

        # per-batch staged tiles (created lazily, rotated by pool bufs=2)
        stage = {}

        def get_stage(b):
            if b not in stage:
                stage[b] = {
                    "qcat": [qkpool.tile([P, S], F32R, tag=f"qcat{h}",
                                         name=f"qcat{h}") for h in range(HPC)],
                    "kcr": [qkpool.tile([P, S], F32R, tag=f"kcr{h}",
                                        name=f"kcr{h}") for h in range(HPC)],
                    "kci": [qkpool.tile([P, S], F32R, tag=f"kci{h}",
                                        name=f"kci{h}") for h in range(HPC)],
                    "va": [vpool.tile([P, S], BF16, tag=f"va{h}",
                                      name=f"va{h}") for h in range(HPC)],
                    "vb": [vpool.tile([P, S], BF16, tag=f"vb{h}",
                                      name=f"vb{h}") for h in range(HPC)],
                    "o": {p: opool.tile([P, S], BF16, tag=f"ost{p}",
                                        name=f"ost{p}") for p in ("r", "i")},
                }
            return stage[b]

        def emit_proj_unit(b, t, half):
            """Gauss projection of one (tensor, 512-token half): 24 mm."""
            st = get_stage(b)
            gt = 2 * b + half
            ps = {}
            for j, part in ((1, "s"), (2, "i"), (3, "r")):
                psj = projps.tile([P, TBLK], F32, tag="projps", name="projps")
                for dc in range(DC):
                    ws = slice(dc * P, (dc + 1) * P)
                    r0 = (dc * NT + gt) * P
                    xt = xpool.tile([P, TBLK], BF16, tag="xt", name="xt")
                    nc.sync.dma_start(xt[:], x_ap[t + part][r0:r0 + P, :])
                    nc.tensor.matmul(psj[:], wt[f"{t}{j}"][:, ws], xt[:],
                                     start=(dc == 0), stop=(dc == DC - 1))
                ps[j] = psj
            # stage T1 to SBUF once; combines read T2/T3 straight from PSUM
            t1sb = t1pool.tile([P, TBLK], F32, tag="t1", name="t1")
            nc.any.tensor_copy(t1sb[:], ps[1][:])
            hs = slice(half * TBLK, (half + 1) * TBLK)
            for h in range(HPC):
                hr = slice(DH * h, DH * (h + 1))
                lo, hi = slice(0, DH), slice(DH, P)
                if t == "q":
                    # qcat[h] = [q_r; q_i]
                    nc.vector.tensor_sub(st["qcat"][h][lo, hs],
                                         t1sb[hr, :], ps[2][hr, :])
                    nc.vector.tensor_add(st["qcat"][h][hi, hs],
                                         t1sb[hr, :], ps[3][hr, :])
                elif t == "k":
                    # kcr = [k_r; -k_i], kci = [k_i; k_r]
                    nc.vector.tensor_sub(st["kcr"][h][lo, hs],
                                         t1sb[hr, :], ps[2][hr, :])
                    nc.vector.tensor_sub(st["kci"][h][hi, hs],
                                         t1sb[hr, :], ps[2][hr, :])
                    nc.vector.tensor_add(st["kci"][h][lo, hs],
                                         t1sb[hr, :], ps[3][hr, :])
                    # -k_i = (t1 * -1) - t3 in one fused DVE op
                    nc.vector.scalar_tensor_tensor(
                        st["kcr"][h][hi, hs], t1sb[hr, :], -1.0, ps[3][hr, :],
                        AluOpType.mult, AluOpType.subtract)
                else:
                    vst = vstpool.tile([P, TBLK], BF16, tag="vst", name="vst")
                    nc.vector.tensor_sub(vst[lo, :], t1sb[hr, :], ps[2][hr, :])
                    nc.vector.tensor_add(vst[hi, :], t1sb[hr, :], ps[3][hr, :])
                    ptb = sps.tile([P, TBLK], BF16, tag="sps", name="ptb")
                    for blk in range(4):
                        bs = slice(blk * P, (blk + 1) * P)
                        nc.tensor.transpose(ptb[:, bs], vst[:, bs], ident[:])
                    # ptb cols per blk: [v_r(h) 64 | v_i(h) 64]
                    base = half * TBLK
                    nc.vector.tensor_copy(st["va"][h][:, base:base + TBLK],
                                          ptb[:])
                    vbv = st["vb"][h][:, base:base + TBLK].rearrange(
                        "p (k c) -> p k c", c=P)
                    ptv = ptb[:].rearrange("p (k c) -> p k c", c=P)
                    nc.vector.tensor_copy(vbv[:, :, 0:DH], ptv[:, :, DH:P])
                    nc.vector.tensor_copy(vbv[:, :, DH:P], ptv[:, :, 0:DH])

        def emit_attn_group(b, h, qb):
            """One (head, 512-query block): 32 score/AV matmuls + 2 Z."""
            st = get_stage(b)
            qs = slice(qb * TBLK, (qb + 1) * TBLK)
            ota = ops_pool.tile([P, TBLK], F32, tag="ota", name="ota")
            otb = ops_pool.tile([P, TBLK], F32, tag="otb", name="otb")
            acc = {"r": [], "i": []}  # pairwise tree partials

            def tree_push(part, t_new):
                lst = acc[part]
                lst.append((0, t_new))
                while len(lst) >= 2 and lst[-1][0] == lst[-2][0]:
                    r1, a = lst.pop()
                    _, bt = lst.pop()
                    s = uspool.tile([P, TBLK], BF16, tag=f"us{part}",
                                    name=f"us{part}")
                    nc.vector.tensor_add(s[:], a[:], bt[:])
                    lst.append((r1 + 1, s))

            for kc in range(KC):
                ks = slice(kc * P, (kc + 1) * P)
                first, last = kc == 0, kc == KC - 1
                str_ = sps.tile([P, TBLK], F32, tag="sps", name="sps")
                nc.tensor.matmul(str_[:], st["kcr"][h][:, ks],
                                 st["qcat"][h][:, qs], start=True, stop=True)
                ur = upool.tile([P, TBLK], BF16, tag="u", name="u")
                nc.scalar.activation(ur[:], str_[:], EXP)
                sti = sps.tile([P, TBLK], F32, tag="sps", name="sps")
                nc.tensor.matmul(sti[:], st["kci"][h][:, ks],
                                 st["qcat"][h][:, qs], start=True, stop=True)
                ui = upool.tile([P, TBLK], BF16, tag="u", name="u")
                nc.scalar.activation(ui[:], sti[:], EXP)
                nc.tensor.matmul(ota[:], st["va"][h][:, ks], ur[:],
                                 start=first, stop=last)
                nc.tensor.matmul(otb[:], st["vb"][h][:, ks], ui[:],
                                 start=first, stop=last)
                tree_push("r", ur)
                tree_push("i", ui)
            usum = {}
            for part in ("r", "i"):
                lst = acc[part]
                while len(lst) >= 2:  # KC is a power of 2, but be safe
                    _, a = lst.pop()
                    _, bt = lst.pop()
                    s = uspool.tile([P, TBLK], BF16, tag=f"us{part}",
                                    name=f"us{part}")
                    nc.vector.tensor_add(s[:], a[:], bt[:])
                    lst.append((99, s))
                usum[part] = lst[0][1]
            # Z replicated across partitions via one ones-matmul per part;
            # each AV term gets its OWN denominator (independent softmaxes).
            # Z psums come from the sps ring: no shared-bank serialization.
            zinv = {}
            for part in ("r", "i"):
                zps = sps.tile([P, TBLK], F32, tag="sps", name="zsum")
                nc.tensor.matmul(zps[:], ones[:], usum[part][:],
                                 start=True, stop=True)
                zinv[part] = zpool.tile([P, TBLK], F32, tag="zinv",
                                        name=f"zinv{part}")
                nc.vector.reciprocal_approx_fast(zinv[part][:], zps[:])
            tmpa = tmppool.tile([P, TBLK], F32, tag="tmp", name="tmpa")
            nc.vector.tensor_mul(tmpa[:], ota[:], zinv["r"][:])
            tmpb = tmppool.tile([P, TBLK], F32, tag="tmp", name="tmpb")
            nc.vector.tensor_mul(tmpb[:], otb[:], zinv["i"][:])
            dst = slice(DH * h, DH * (h + 1))
            nc.vector.tensor_sub(st["o"]["r"][dst, qs], tmpa[0:DH, :],
                                 tmpb[0:DH, :])
            nc.vector.tensor_add(st["o"]["i"][dst, qs], tmpa[DH:P, :],
                                 tmpb[DH:P, :])

        def emit_oproj(b, half):
            """Partial O-projection for one 512-token half: 32 matmuls."""
            st = get_stage(b)
            hs = slice(half * TBLK, (half + 1) * TBLK)
            gt = 2 * b + half
            for mc in range(DC):
                ms = slice(mc * P, (mc + 1) * P)
                orow = (gt * DC + mc) * P
                pr = sps.tile([P, TBLK], F32, tag="sps", name="ojpr")
                nc.tensor.matmul(pr[:], wot["r"][:, ms], st["o"]["r"][:, hs],
                                 start=True, stop=False)
                nc.tensor.matmul(pr[:], wot["in"][:, ms], st["o"]["i"][:, hs],
                                 start=False, stop=True)
                sbr = popool.tile([P, TBLK], BF16, tag="po", name="po")
                nc.any.tensor_copy(sbr[:], pr[:])
                nc.sync.dma_start(po_r[orow:orow + P, :], sbr[:])
                pi = sps.tile([P, TBLK], F32, tag="sps", name="ojpi")
                nc.tensor.matmul(pi[:], wot["i"][:, ms], st["o"]["r"][:, hs],
                                 start=True, stop=False)
                nc.tensor.matmul(pi[:], wot["r"][:, ms], st["o"]["i"][:, hs],
                                 start=False, stop=True)
                sbi = popool.tile([P, TBLK], BF16, tag="po", name="po")
                nc.any.tensor_copy(sbi[:], pi[:])
                nc.sync.dma_start(po_i[orow:orow + P, :], sbi[:])

        # ---- prologue: weights in first-use order, batch-0 projections
        # start after only the q-weights are queued (~1MB of DMA lead-in).
        for j in (1, 2, 3):
            load_w(f"q{j}")
        ident = wpool.tile([P, P], BF16, tag="ident", name="ident")
        nc.sync.dma_start(ident[:], ident_ap[:])
        ones = wpool.tile([P, P], BF16, tag="ones", name="ones")
        nc.sync.dma_start(ones[:], ones_ap[:])
        emit_proj_unit(0, "q", 0)
        for j in (1, 2, 3):
            load_w(f"k{j}")
        emit_proj_unit(0, "q", 1)
        emit_proj_unit(0, "k", 0)
        for j in (1, 2, 3):
            load_w(f"v{j}")
        emit_proj_unit(0, "k", 1)
        emit_proj_unit(0, "v", 0)
        wot = {}
        for suf, ap in wo_ap.items():
            wot[suf] = wpool.tile([P, D], BF16, tag=f"wo_{suf}",
                                  name=f"wo_{suf}")
            nc.sync.dma_start(wot[suf][:], ap[:])
        emit_proj_unit(0, "v", 1)

        # ---- steady state: attention(b) interleaved with projection(b+1)
        # and oproj(b) so the PE queue never drains across group-end
        # dependency bubbles. Group order (0,0),(1,0) completes the qb=0
        # half of o_stage early so oproj(b,0) becomes mid-batch PE filler.
        for b in range(B):
            nxt = b + 1
            emit_attn_group(b, 0, 0)
            if nxt < B:
                emit_proj_unit(nxt, "q", 0)
            emit_attn_group(b, 1, 0)
            emit_oproj(b, 0)
            if nxt < B:
                emit_proj_unit(nxt, "q", 1)
                emit_proj_unit(nxt, "k", 0)
            emit_attn_group(b, 0, 1)
            if nxt < B:
                emit_proj_unit(nxt, "k", 1)
                emit_proj_unit(nxt, "v", 0)
            emit_attn_group(b, 1, 1)
            emit_oproj(b, 1)
            if nxt < B:
                emit_proj_unit(nxt, "v", 1)
            stage.pop(b, None)

    nc.compile()
    return nc


def _w_sbuf_layout(w_t):
    """[D, 128] weight-transpose slice -> SBUF layout [128, dc*128+o]."""
    return np.ascontiguousarray(
        w_t.reshape(DC, P, P).transpose(1, 0, 2).reshape(P, D))


def _tile_x(xT, dtype):
    """[D, B*S] -> tiled-contiguous [DC*NT*P, TBLK] (rows: (dc*NT+gt)*P)."""
    t = xT.reshape(DC, P, NT, TBLK).transpose(0, 2, 1, 3)
    return np.ascontiguousarray(t.reshape(DC * NT * P, TBLK)).astype(dtype)


def _prepare_in_maps(inputs):
    bf = ml_dtypes.bfloat16
    xs = {}
    for name, t in (("queries", "q"), ("keys", "k"), ("values", "v")):
        x = np.asarray(inputs[name], dtype=np.float32)  # [B,S,D,2]
        flat = x.reshape(B * S, D, 2)
        xs[t + "r"] = _tile_x(flat[:, :, 0].T, bf)
        xs[t + "i"] = _tile_x(flat[:, :, 1].T, bf)
        xs[t + "s"] = _tile_x((flat[:, :, 0] + flat[:, :, 1]).T, bf)

    scale = np.float32(1.0 / np.sqrt(DH))
    in_maps = []
    for c in range(NCORES):
        rows = slice(P * c, P * (c + 1))
        m = {}
        for t in ("q", "k", "v"):
            for part in ("r", "i", "s"):
                m[f"x{t}_{part}"] = xs[t + part]
        for t, wr_name, wi_name in (("q", "wq_r", "wq_i"),
                                    ("k", "wk_r", "wk_i"),
                                    ("v", "wv_r", "wv_i")):
            s = scale if t == "q" else np.float32(1.0)
            wr = np.asarray(inputs[wr_name], dtype=np.float32)[rows] * s
            wi = np.asarray(inputs[wi_name], dtype=np.float32)[rows] * s
            # Gauss packs, both heads in M: [64 cols h0 | 64 cols h1]
            packs = {1: wr, 2: wr + wi, 3: wi - wr}
            for j, w in packs.items():
                wcat = np.concatenate(
                    [w[DH * h:DH * (h + 1)].T for h in range(HPC)], axis=1)
                m[f"w{t}_{j}"] = _w_sbuf_layout(wcat).astype(bf)
        wo_r = np.asarray(inputs["wo_r"], dtype=np.float32)[:, rows]  # [D,128]
        wo_i = np.asarray(inputs["wo_i"], dtype=np.float32)[:, rows]
        m["wo_r"] = np.ascontiguousarray(wo_r.T).astype(bf)  # [128 d, 1024 m]
        m["wo_i"] = np.ascontiguousarray(wo_i.T).astype(bf)
        m["wo_in"] = np.ascontiguousarray(-wo_i.T).astype(bf)
        m["ident"] = np.eye(P, dtype=bf)
        m["onesin"] = np.ones((P, P), dtype=bf)
        in_maps.append(m)
    return in_maps


LAST_RESULT = None


def _run(inputs, trace=False):
    global LAST_RESULT
    from concourse.bass_utils import run_bass_kernel_spmd
    if "nc" not in _CACHE:
        _CACHE["nc"] = _build()
    nc = _CACHE["nc"]
    in_maps = _prepare_in_maps(inputs)
    if trace:
        os.environ.pop("BASS_NEVER_TRACE", None)
    else:
        os.environ["BASS_NEVER_TRACE"] = "1"
    res = run_bass_kernel_spmd(nc, in_maps, core_ids=list(range(NCORES)),
                               trace=trace)
    LAST_RESULT = res
    acc_r = np.zeros((NT * DC * P, TBLK), np.float32)
    acc_i = np.zeros((NT * DC * P, TBLK), np.float32)
    for c in range(NCORES):
        acc_r += res.results[c]["po_r"].astype(np.float32)
        acc_i += res.results[c]["po_i"].astype(np.float32)

    def untile(po):
        # [NT*DC*P, TBLK] rows (gt*DC+mc)*P -> [D, B*S] -> [B,S,D]
        t = po.reshape(NT, DC, P, TBLK).transpose(1, 2, 0, 3)
        return np.ascontiguousarray(t.reshape(D, B * S)).T.reshape(B, S, D)

    out = np.empty((B, S, D, 2), np.float32)
    out[..., 0] = untile(acc_r)
    out[..., 1] = untile(acc_i)
    return out


def kernel(**inputs):
    return _run(inputs, trace=False)


# revision 5
# speedup vs baseline: 1.1869x; 1.1869x over previous
"""ComplexMultiHeadAttention on 8 TRN2 NeuronCores (Bass/Tile).

Problem: B=4, S=1024, D_MODEL=1024, N_HEADS=16, D_HEAD=64, complex-valued
activations stored as a trailing dim of size 2 (real, imag).

    q = to_heads(complex_linear(queries, wq));  k, v likewise
    s_r + i*s_i = (q_r + i q_i)(k_r + i k_i)^T / sqrt(dh)
    a_r = softmax(s_r), a_i = softmax(s_i)      (independent softmaxes)
    o = complex_bmm(a, v);  out = complex_linear(concat_heads(o), wo)

Sharding: head-parallel. Core c owns heads {2c, 2c+1} = 128 contiguous dims
of the hidden axis. Each core computes Q/K/V projections for its 128 output
dims (weights row-sliced), runs attention for its 8 (batch, head) pairs, and
computes a partial O-projection (wo column-sliced on its 128 input dims)
over all 1024 output dims. The host sums the 8 partial outputs — no
on-device collectives.

Layout: tokens always on the FREE dim, features/keys on partitions, so
every matmul is a natural lhsT.T @ rhs with K=128 contraction:
  - inputs passed transposed: x^T [1024 d, 4096 t] (bf16; halves DMA).
    Each x tile feeds TWO matmuls (psr+psi) — this keeps the projection
    phase PE-bound, not DMA-bound (a 128KB tile takes ~356ns to land,
    two matmuls take ~426ns of PE). A 3-stream Gauss variant that fed
    one matmul per tile measured SLOWER (DMA-starved PE, p-state drops).
  - projections produce psum [128 outdims, 512 t]; the complex parts are
    handled by accumulating with sign-folded weight copies (w_i, -w_i).
  - scores are computed TRANSPOSED (s^T [k, q]) from Qcat = [q_r; q_i],
    Kcat_r = [k_r; -k_i], Kcat_i = [k_i; k_r] (all [128, S] f32r) — one
    K=128 matmul per 128-key chunk, no accumulation.
  - softmax over k (= partitions) skips max-subtraction (scores are O(1)
    by construction, exp cannot overflow). exp writes bf16 u-tiles; the
    8 per-part u tiles are pairwise tree-summed on DVE (7 bf16 adds) and
    ONE ones[128,128]-matmul replicates the total Z across partitions,
    making the 1/Z scale an aligned tensor_mul. Z psums allocate from the
    score-psum ring (sps) so the two Z matmuls of a group land in
    different banks — no recip->matmul serialization.
  - V is PE-transposed into token-major packs VA=[v_r|v_i], VB=[v_i|v_r],
    so attn@V accumulates o_pack [o_r|o_i, q] in a single psum group.
  - the O-projection uses Gauss's 3-multiplication complex product
    (T1=wo_r@(o_r+o_i), T2=(wo_r+wo_i)@o_i, T3=(wo_i-wo_r)@o_r;
    out_r=T1-T2, out_i=T1+T3): 3 matmuls per output chunk instead of 4.
    Its operands are SBUF-resident so the extra (o_r+o_i) stream costs
    one DVE add per half, not DMA.

Scheduling: the PE p-state drops (~2x slower matmuls) whenever the engine
idles, so projection work for batch b+1 and the O-projections of batch b
are interleaved between attention groups of batch b to keep the PE queue
non-empty across group-end dependency bubbles. Weights load in first-use
order so the first matmul starts early instead of waiting on all weight
DMA.

Matmul dtype note (cost-model + HW verified): bf16 and f32r both run at
1 cycle/row for >=256-row moving operands, so dtype choice is about DMA
bytes and precision, not PE speed. Scores/Q/K stay f32r in SBUF (exp is
the error-sensitive consumer); x/weights/u/V/outputs are bf16.
"""

import os
import numpy as np
import ml_dtypes
from contextlib import ExitStack

import concourse.bass as bass
import concourse.tile as tile
from concourse import bacc, mybir
from concourse.alu_op_type import AluOpType

F32 = mybir.dt.float32
F32R = mybir.dt.float32r
BF16 = mybir.dt.bfloat16
EXP = mybir.ActivationFunctionType.Exp

B, S, D, H, DH = 4, 1024, 1024, 16, 64
NCORES = 8
P = 128            # partitions / chunk size
TBLK = 512         # token block (matmul free dim)
DC = D // P        # 8 d-chunks
KC = S // P        # 8 key chunks per batch
HPC = H // NCORES  # 2 heads per core
NT = (B * S) // TBLK  # 8 token blocks

_CACHE = {}


def _build():
    nc = bacc.Bacc("TRN2", target_bir_lowering=False, debug=False,
                   num_devices=NCORES)

    x_ap = {}
    for t in ("q", "k", "v"):
        for part in ("r", "i"):
            # tiled-contiguous layout: row block (dc*NT + gt)*P : +P is one
            # [128, 512] tile stored contiguously (single-descriptor DMA)
            x_ap[t + part] = nc.dram_tensor(
                f"x{t}_{part}", [DC * NT * P, TBLK],
                BF16, kind="ExternalInput").ap()
    # all projections use per-head combined weights: one psum directly
    # produces the attention layout ([q_r;q_i], [k_r;-k_i], [v_r;v_i])
    w_ap = {}
    for t in ("q", "k", "v"):
        for h in range(HPC):
            for suf in ("a", "b"):
                w_ap[f"{t}{suf}{h}"] = nc.dram_tensor(
                    f"w{t}_{suf}{h}", [P, D], BF16, kind="ExternalInput").ap()
    # O-projection Gauss packs: w1=wo_r, w2=wo_r+wo_i, w3=wo_i-wo_r
    wo_ap = {}
    for j in (1, 2, 3):
        wo_ap[j] = nc.dram_tensor(
            f"wo_{j}", [P, D], BF16, kind="ExternalInput").ap()
    ident_ap = nc.dram_tensor("ident", [P, P], BF16, kind="ExternalInput").ap()
    ones_ap = nc.dram_tensor("onesin", [P, P], BF16, kind="ExternalInput").ap()
    # same tiled-contiguous trick for outputs: row block (gt*DC + mc)*P
    po_r = nc.dram_tensor("po_r", [NT * DC * P, TBLK], BF16,
                          kind="ExternalOutput").ap()
    po_i = nc.dram_tensor("po_i", [NT * DC * P, TBLK], BF16,
                          kind="ExternalOutput").ap()

    with tile.TileContext(nc) as tc, ExitStack() as ctx:
        wpool = ctx.enter_context(tc.tile_pool(name="w", bufs=1))
        xpool = ctx.enter_context(tc.tile_pool(name="x", bufs=20))
        qkpool = ctx.enter_context(tc.tile_pool(name="qk", bufs=2))
        vpool = ctx.enter_context(tc.tile_pool(name="v", bufs=2))
        opool = ctx.enter_context(tc.tile_pool(name="ost", bufs=2))
        ospool = ctx.enter_context(tc.tile_pool(name="osum", bufs=2))
        upool = ctx.enter_context(tc.tile_pool(name="u", bufs=8))
        uspool = ctx.enter_context(tc.tile_pool(name="us", bufs=8))
        t1pool = ctx.enter_context(tc.tile_pool(name="t1", bufs=2))
        zpool = ctx.enter_context(tc.tile_pool(name="z", bufs=2))
        tmppool = ctx.enter_context(tc.tile_pool(name="tmp", bufs=4))
        popool = ctx.enter_context(tc.tile_pool(name="po", bufs=4))
        vstpool = ctx.enter_context(tc.tile_pool(name="vst", bufs=2))
        # PSUM: 8 banks. projps (2) ping-pongs the psr/psi accumulators;
        # sps (4) rotates score tiles, Z sums, the V-transpose target and
        # the O-projection Gauss accumulators; ops 2 (ota+otb).
        projps = ctx.enter_context(tc.tile_pool(name="pp", bufs=2, space="PSUM"))
        sps = ctx.enter_context(tc.tile_pool(name="sp", bufs=4, space="PSUM"))
        ops_pool = ctx.enter_context(tc.tile_pool(name="op", bufs=1, space="PSUM"))

        wt = {}

        def load_w(key):
            wt[key] = wpool.tile([P, D], BF16, tag=f"w_{key}", name=f"w_{key}")
            nc.sync.dma_start(wt[key][:], w_ap[key][:])

        # per-batch staged tiles (created lazily, rotated by pool bufs=2)
        stage = {}

        def get_stage(b):
            if b not in stage:
                stage[b] = {
                    "qcat": [qkpool.tile([P, S], F32R, tag=f"qcat{h}",
                                         name=f"qcat{h}") for h in range(HPC)],
                    "kcr": [qkpool.tile([P, S], F32R, tag=f"kcr{h}",
                                        name=f"kcr{h}") for h in range(HPC)],
                    "kci": [qkpool.tile([P, S], F32R, tag=f"kci{h}",
                                        name=f"kci{h}") for h in range(HPC)],
                    "va": [vpool.tile([P, S], BF16, tag=f"va{h}",
                                      name=f"va{h}") for h in range(HPC)],
                    "vb": [vpool.tile([P, S], BF16, tag=f"vb{h}",
                                      name=f"vb{h}") for h in range(HPC)],
                    "o": {p: opool.tile([P, S], BF16, tag=f"ost{p}",
                                        name=f"ost{p}") for p in ("r", "i")},
                    "osum": ospool.tile([P, S], BF16, tag="osum",
                                        name="osum"),
                }
            return stage[b]

        def emit_proj_unit(b, t, half):
            """Projection of one (tensor, 512-token half): 32 matmuls,
            each DMA'd x tile feeding two of them."""
            st = get_stage(b)
            gt = 2 * b + half
            wA = (wt[t + "a0"], wt[t + "a1"])
            wB = (wt[t + "b0"], wt[t + "b1"])
            psr = projps.tile([P, TBLK], F32, tag="projps", name="projps")
            psi = projps.tile([P, TBLK], F32, tag="projps", name="projps")
            for dc in range(DC):
                ws = slice(dc * P, (dc + 1) * P)
                r0 = (dc * NT + gt) * P
                xrt = xpool.tile([P, TBLK], BF16, tag="xt", name="xt")
                nc.sync.dma_start(xrt[:], x_ap[t + "r"][r0:r0 + P, :])
                nc.tensor.matmul(psr[:], wA[0][:, ws], xrt[:],
                                 start=(dc == 0), stop=False)
                nc.tensor.matmul(psi[:], wA[1][:, ws], xrt[:],
                                 start=(dc == 0), stop=False)
            for dc in range(DC):
                ws = slice(dc * P, (dc + 1) * P)
                r0 = (dc * NT + gt) * P
                xit = xpool.tile([P, TBLK], BF16, tag="xt", name="xt")
                nc.sync.dma_start(xit[:], x_ap[t + "i"][r0:r0 + P, :])
                nc.tensor.matmul(psr[:], wB[0][:, ws], xit[:],
                                 start=False, stop=(dc == DC - 1))
                nc.tensor.matmul(psi[:], wB[1][:, ws], xit[:],
                                 start=False, stop=(dc == DC - 1))
            hs = slice(half * TBLK, (half + 1) * TBLK)
            if t == "q":
                # psX = [q_r(h); q_i(h)] = Qcat directly
                for h, psx in ((0, psr), (1, psi)):
                    nc.vector.tensor_copy(st["qcat"][h][:, hs], psx[:])
            elif t == "k":
                # psX = [k_r(h); -k_i(h)] = Kcat_r directly;
                # Kcat_i = [k_i; k_r] via one negate + one copy
                for h, psx in ((0, psr), (1, psi)):
                    nc.vector.tensor_copy(st["kcr"][h][:, hs], psx[:])
                    nc.vector.tensor_scalar_mul(st["kci"][h][0:DH, hs],
                                                psx[DH:P, :], -1.0)
                    nc.vector.tensor_copy(st["kci"][h][DH:P, hs],
                                          psx[0:DH, :])
            else:
                # psr = [v_r(h0); v_i(h0)], psi = [v_r(h1); v_i(h1)]
                for h, psx in ((0, psr), (1, psi)):
                    vst = vstpool.tile([P, TBLK], BF16, tag="vst", name="vst")
                    nc.vector.tensor_copy(vst[:], psx[:])
                    ptb = sps.tile([P, TBLK], BF16, tag="sps", name="ptb")
                    for blk in range(4):
                        bs = slice(blk * P, (blk + 1) * P)
                        nc.tensor.transpose(ptb[:, bs], vst[:, bs], ident[:])
                    # ptb cols per blk: [v_r(h) 64 | v_i(h) 64]
                    base = half * TBLK
                    nc.vector.tensor_copy(st["va"][h][:, base:base + TBLK],
                                          ptb[:])
                    vbv = st["vb"][h][:, base:base + TBLK].rearrange(
                        "p (k c) -> p k c", c=P)
                    ptv = ptb[:].rearrange("p (k c) -> p k c", c=P)
                    nc.vector.tensor_copy(vbv[:, :, 0:DH], ptv[:, :, DH:P])
                    nc.vector.tensor_copy(vbv[:, :, DH:P], ptv[:, :, 0:DH])

        def emit_attn_group(b, h, qb):
            """One (head, 512-query block): 32 score/AV matmuls + 2 Z."""
            st = get_stage(b)
            qs = slice(qb * TBLK, (qb + 1) * TBLK)
            ota = ops_pool.tile([P, TBLK], F32, tag="ota", name="ota")
            otb = ops_pool.tile([P, TBLK], F32, tag="otb", name="otb")
            acc = {"r": [], "i": []}  # pairwise tree partials

            def tree_push(part, t_new):
                lst = acc[part]
                lst.append((0, t_new))
                while len(lst) >= 2 and lst[-1][0] == lst[-2][0]:
                    r1, a = lst.pop()
                    _, bt = lst.pop()
                    s = uspool.tile([P, TBLK], BF16, tag=f"us{part}",
                                    name=f"us{part}")
                    nc.vector.tensor_add(s[:], a[:], bt[:])
                    lst.append((r1 + 1, s))

            for kc in range(KC):
                ks = slice(kc * P, (kc + 1) * P)
                first, last = kc == 0, kc == KC - 1
                str_ = sps.tile([P, TBLK], F32, tag="sps", name="sps")
                nc.tensor.matmul(str_[:], st["kcr"][h][:, ks],
                                 st["qcat"][h][:, qs], start=True, stop=True)
                ur = upool.tile([P, TBLK], BF16, tag="u", name="u")
                nc.scalar.activation(ur[:], str_[:], EXP)
                sti = sps.tile([P, TBLK], F32, tag="sps", name="sps")
                nc.tensor.matmul(sti[:], st["kci"][h][:, ks],
                                 st["qcat"][h][:, qs], start=True, stop=True)
                ui = upool.tile([P, TBLK], BF16, tag="u", name="u")
                nc.scalar.activation(ui[:], sti[:], EXP)
                nc.tensor.matmul(ota[:], st["va"][h][:, ks], ur[:],
                                 start=first, stop=last)
                nc.tensor.matmul(otb[:], st["vb"][h][:, ks], ui[:],
                                 start=first, stop=last)
                tree_push("r", ur)
                tree_push("i", ui)
            usum = {}
            for part in ("r", "i"):
                lst = acc[part]
                while len(lst) >= 2:  # KC is a power of 2, but be safe
                    _, a = lst.pop()
                    _, bt = lst.pop()
                    s = uspool.tile([P, TBLK], BF16, tag=f"us{part}",
                                    name=f"us{part}")
                    nc.vector.tensor_add(s[:], a[:], bt[:])
                    lst.append((99, s))
                usum[part] = lst[0][1]
            # Z replicated across partitions via one ones-matmul per part;
            # each AV term gets its OWN denominator (independent softmaxes).
            # Z psums come from the sps ring: no shared-bank serialization.
            zinv = {}
            for part in ("r", "i"):
                zps = sps.tile([P, TBLK], F32, tag="sps", name="zsum")
                nc.tensor.matmul(zps[:], ones[:], usum[part][:],
                                 start=True, stop=True)
                zinv[part] = zpool.tile([P, TBLK], F32, tag="zinv",
                                        name=f"zinv{part}")
                nc.vector.reciprocal_approx_fast(zinv[part][:], zps[:])
            tmpa = tmppool.tile([P, TBLK], F32, tag="tmp", name="tmpa")
            nc.vector.tensor_mul(tmpa[:], ota[:], zinv["r"][:])
            tmpb = tmppool.tile([P, TBLK], F32, tag="tmp", name="tmpb")
            nc.vector.tensor_mul(tmpb[:], otb[:], zinv["i"][:])
            dst = slice(DH * h, DH * (h + 1))
            nc.vector.tensor_sub(st["o"]["r"][dst, qs], tmpa[0:DH, :],
                                 tmpb[0:DH, :])
            nc.vector.tensor_add(st["o"]["i"][dst, qs], tmpa[DH:P, :],
                                 tmpb[DH:P, :])
            # feed the O-projection Gauss stream as soon as both halves of
            # this (qb, h-pair) are done; h==1 closes the pair
            if h == 1:
                nc.vector.tensor_add(st["osum"][:, qs], st["o"]["r"][:, qs],
                                     st["o"]["i"][:, qs])

        def emit_oproj(b, half):
            """Partial O-projection (Gauss, 3 mm/chunk) for one half."""
            st = get_stage(b)
            hs = slice(half * TBLK, (half + 1) * TBLK)
            gt = 2 * b + half
            for mc in range(DC):
                ms = slice(mc * P, (mc + 1) * P)
                orow = (gt * DC + mc) * P
                t1 = sps.tile([P, TBLK], F32, tag="sps", name="ojt1")
                nc.tensor.matmul(t1[:], wot[1][:, ms], st["osum"][:, hs],
                                 start=True, stop=True)
                t2 = sps.tile([P, TBLK], F32, tag="sps", name="ojt2")
                nc.tensor.matmul(t2[:], wot[2][:, ms], st["o"]["i"][:, hs],
                                 start=True, stop=True)
                t3 = sps.tile([P, TBLK], F32, tag="sps", name="ojt3")
                nc.tensor.matmul(t3[:], wot[3][:, ms], st["o"]["r"][:, hs],
                                 start=True, stop=True)
                t1c = t1pool.tile([P, TBLK], F32, tag="t1", name="t1")
                nc.any.tensor_copy(t1c[:], t1[:])
                sbr = popool.tile([P, TBLK], BF16, tag="po", name="po")
                nc.vector.tensor_sub(sbr[:], t1c[:], t2[:])
                nc.sync.dma_start(po_r[orow:orow + P, :], sbr[:])
                sbi = popool.tile([P, TBLK], BF16, tag="po", name="po")
                nc.vector.tensor_add(sbi[:], t1c[:], t3[:])
                nc.sync.dma_start(po_i[orow:orow + P, :], sbi[:])

        # ---- prologue: weights in first-use order, batch-0 projections
        # start after only the q-weights are queued.
        for h in range(HPC):
            for suf in ("a", "b"):
                load_w(f"q{suf}{h}")
        ident = wpool.tile([P, P], BF16, tag="ident", name="ident")
        nc.sync.dma_start(ident[:], ident_ap[:])
        ones = wpool.tile([P, P], BF16, tag="ones", name="ones")
        nc.sync.dma_start(ones[:], ones_ap[:])
        emit_proj_unit(0, "q", 0)
        for h in range(HPC):
            for suf in ("a", "b"):
                load_w(f"k{suf}{h}")
        emit_proj_unit(0, "q", 1)
        emit_proj_unit(0, "k", 0)
        for h in range(HPC):
            for suf in ("a", "b"):
                load_w(f"v{suf}{h}")
        emit_proj_unit(0, "k", 1)
        emit_proj_unit(0, "v", 0)
        wot = {}
        for j, ap in wo_ap.items():
            wot[j] = wpool.tile([P, D], BF16, tag=f"wo_{j}", name=f"wo_{j}")
            nc.sync.dma_start(wot[j][:], ap[:])
        emit_proj_unit(0, "v", 1)

        # ---- steady state: attention(b) interleaved with projection(b+1)
        # and oproj(b) so the PE queue never drains across group-end
        # dependency bubbles. Group order (0,0),(1,0) completes the qb=0
        # half of o_stage early so oproj(b,0) becomes mid-batch PE filler.
        for b in range(B):
            nxt = b + 1
            emit_attn_group(b, 0, 0)
            if nxt < B:
                emit_proj_unit(nxt, "q", 0)
            emit_attn_group(b, 1, 0)
            emit_oproj(b, 0)
            if nxt < B:
                emit_proj_unit(nxt, "q", 1)
                emit_proj_unit(nxt, "k", 0)
            emit_attn_group(b, 0, 1)
            if nxt < B:
                emit_proj_unit(nxt, "k", 1)
                emit_proj_unit(nxt, "v", 0)
            emit_attn_group(b, 1, 1)
            emit_oproj(b, 1)
            if nxt < B:
                emit_proj_unit(nxt, "v", 1)
            stage.pop(b, None)

    nc.compile()
    return nc


def _w_sbuf_layout(w_t):
    """[D, 128] weight-transpose slice -> SBUF layout [128, dc*128+o]."""
    return np.ascontiguousarray(
        w_t.reshape(DC, P, P).transpose(1, 0, 2).reshape(P, D))


def _tile_x(xT, dtype):
    """[D, B*S] -> tiled-contiguous [DC*NT*P, TBLK] (rows: (dc*NT+gt)*P)."""
    t = xT.reshape(DC, P, NT, TBLK).transpose(0, 2, 1, 3)
    return np.ascontiguousarray(t.reshape(DC * NT * P, TBLK)).astype(dtype)


def _prepare_in_maps(inputs):
    bf = ml_dtypes.bfloat16
    xs = {}
    for name, t in (("queries", "q"), ("keys", "k"), ("values", "v")):
        x = np.asarray(inputs[name], dtype=np.float32)  # [B,S,D,2]
        flat = x.reshape(B * S, D, 2)
        xs[t + "r"] = _tile_x(flat[:, :, 0].T, bf)
        xs[t + "i"] = _tile_x(flat[:, :, 1].T, bf)

    scale = np.float32(1.0 / np.sqrt(DH))
    in_maps = []
    for c in range(NCORES):
        rows = slice(P * c, P * (c + 1))
        m = {}
        for t in ("q", "k", "v"):
            for part in ("r", "i"):
                m[f"x{t}_{part}"] = xs[t + part]
        for t, wr_name, wi_name in (("q", "wq_r", "wq_i"),
                                    ("k", "wk_r", "wk_i"),
                                    ("v", "wv_r", "wv_i")):
            s = scale if t == "q" else np.float32(1.0)
            wr = np.asarray(inputs[wr_name], dtype=np.float32)[rows] * s
            wi = np.asarray(inputs[wi_name], dtype=np.float32)[rows] * s
            for h in range(HPC):
                hr = slice(DH * h, DH * (h + 1))
                if t == "q":
                    wa = np.concatenate([wr[hr].T, wi[hr].T], axis=1)
                    wb = np.concatenate([-wi[hr].T, wr[hr].T], axis=1)
                elif t == "k":
                    wa = np.concatenate([wr[hr].T, -wi[hr].T], axis=1)
                    wb = np.concatenate([-wi[hr].T, -wr[hr].T], axis=1)
                else:
                    wa = np.concatenate([wr[hr].T, wi[hr].T], axis=1)
                    wb = np.concatenate([-wi[hr].T, wr[hr].T], axis=1)
                m[f"w{t}_a{h}"] = _w_sbuf_layout(wa).astype(bf)
                m[f"w{t}_b{h}"] = _w_sbuf_layout(wb).astype(bf)
        wo_r = np.asarray(inputs["wo_r"], dtype=np.float32)[:, rows]  # [D,128]
        wo_i = np.asarray(inputs["wo_i"], dtype=np.float32)[:, rows]
        # Gauss packs for the O-projection
        m["wo_1"] = np.ascontiguousarray(wo_r.T).astype(bf)  # [128 d, 1024 m]
        m["wo_2"] = np.ascontiguousarray((wo_r + wo_i).T).astype(bf)
        m["wo_3"] = np.ascontiguousarray((wo_i - wo_r).T).astype(bf)
        m["ident"] = np.eye(P, dtype=bf)
        m["onesin"] = np.ones((P, P), dtype=bf)
        in_maps.append(m)
    return in_maps


LAST_RESULT = None


def _run(inputs, trace=False):
    global LAST_RESULT
    from concourse.bass_utils import run_bass_kernel_spmd
    if "nc" not in _CACHE:
        _CACHE["nc"] = _build()
    nc = _CACHE["nc"]
    in_maps = _prepare_in_maps(inputs)
    if trace:
        os.environ.pop("BASS_NEVER_TRACE", None)
    else:
        os.environ["BASS_NEVER_TRACE"] = "1"
    res = run_bass_kernel_spmd(nc, in_maps, core_ids=list(range(NCORES)),
                               trace=trace)
    LAST_RESULT = res
    acc_r = np.zeros((NT * DC * P, TBLK), np.float32)
    acc_i = np.zeros((NT * DC * P, TBLK), np.float32)
    for c in range(NCORES):
        acc_r += res.results[c]["po_r"].astype(np.float32)
        acc_i += res.results[c]["po_i"].astype(np.float32)

    def untile(po):
        # [NT*DC*P, TBLK] rows (gt*DC+mc)*P -> [D, B*S] -> [B,S,D]
        t = po.reshape(NT, DC, P, TBLK).transpose(1, 2, 0, 3)
        return np.ascontiguousarray(t.reshape(D, B * S)).T.reshape(B, S, D)

    out = np.empty((B, S, D, 2), np.float32)
    out[..., 0] = untile(acc_r)
    out[..., 1] = untile(acc_i)
    return out


def kernel(**inputs):
    return _run(inputs, trace=False)


# revision 6
# speedup vs baseline: 1.2488x; 1.0521x over previous
"""ComplexMultiHeadAttention on 8 TRN2 NeuronCores (Bass/Tile).

Problem: B=4, S=1024, D_MODEL=1024, N_HEADS=16, D_HEAD=64, complex-valued
activations stored as a trailing dim of size 2 (real, imag).

    q = to_heads(complex_linear(queries, wq));  k, v likewise
    s_r + i*s_i = (q_r + i q_i)(k_r + i k_i)^T / sqrt(dh)
    a_r = softmax(s_r), a_i = softmax(s_i)      (independent softmaxes)
    o = complex_bmm(a, v);  out = complex_linear(concat_heads(o), wo)

Sharding: head-parallel. Core c owns heads {2c, 2c+1} = 128 contiguous dims
of the hidden axis. Each core computes Q/K/V projections for its 128 output
dims (weights row-sliced), runs attention for its 8 (batch, head) pairs, and
computes a partial O-projection (wo column-sliced on its 128 input dims)
over all 1024 output dims. The host sums the 8 partial outputs — no
on-device collectives.

Layout: tokens always on the FREE dim, features/keys on partitions, so
every matmul is a natural lhsT.T @ rhs with K=128 contraction:
  - inputs passed transposed: x^T [1024 d, 4096 t] (bf16; halves DMA).
    Each x tile feeds TWO matmuls (psr+psi) which keeps the projection
    phase PE-bound, not DMA-bound.
  - projections produce psum [128 outdims, 512 t]; the complex parts are
    handled by accumulating with sign-folded weight copies (w_i, -w_i).
  - scores are computed TRANSPOSED (s^T [k, q]) from Qcat = [q_r; q_i],
    Kcat_r = [k_r; -k_i], Kcat_i = [k_i; k_r] (all [128, S] f32r) — one
    K=128 matmul per 128-key chunk, no accumulation.
  - softmax over k (= partitions) skips max-subtraction (scores are O(1)
    by construction, exp cannot overflow). exp writes bf16 u-tiles; the
    8 per-part u tiles are pairwise tree-summed on DVE (7 bf16 adds) and
    ONE ones[128,128]-matmul replicates the total Z across partitions,
    making the 1/Z scale an aligned tensor_mul.
  - V is PE-transposed into token-major packs VA=[v_r|v_i], VB=[v_i|v_r],
    so attn@V accumulates o_pack [o_r|o_i, q] in a single psum group.

Scheduling (the key to p-state): TRN2's PE runs ~2x slower unless it has
been continuously busy for ~3us, so every dependency bubble costs twice.
Projection work for batch b+1 is emitted through GENERATORS that yield
after each 2-matmul step; the attention inner loop pulls one step per
key-chunk and more at group ends, so the PE queue always holds
independent work across the exp-gated score pipeline and group-end
normalize chains. PSUM pools are split by consumer engine so score
matmuls never wait on banks drained by slow queued DVE work:
projps 2 (projection accumulators + V-transpose), sps 2 (scores + Z),
ojps 2 (O-projection accumulators, drained by Scalar copies), ops 2
(AV accumulators). Weights load in first-use order so the first matmul
starts early.

Matmul dtype note (cost-model + HW verified): bf16 and f32r both run at
1 cycle/row for >=256-row moving operands, so dtype choice is about DMA
bytes and precision, not PE speed. Scores/Q/K stay f32r in SBUF (exp is
the error-sensitive consumer); x/weights/u/V/outputs are bf16.
"""

import os
import numpy as np
import ml_dtypes
from collections import deque
from contextlib import ExitStack

import concourse.bass as bass
import concourse.tile as tile
from concourse import bacc, mybir

F32 = mybir.dt.float32
F32R = mybir.dt.float32r
BF16 = mybir.dt.bfloat16
EXP = mybir.ActivationFunctionType.Exp

B, S, D, H, DH = 4, 1024, 1024, 16, 64
NCORES = 8
P = 128            # partitions / chunk size
TBLK = 512         # token block (matmul free dim)
DC = D // P        # 8 d-chunks
KC = S // P        # 8 key chunks per batch
HPC = H // NCORES  # 2 heads per core
NT = (B * S) // TBLK  # 8 token blocks

_CACHE = {}


def _build():
    nc = bacc.Bacc("TRN2", target_bir_lowering=False, debug=False,
                   num_devices=NCORES)

    x_ap = {}
    for t in ("q", "k", "v"):
        for part in ("r", "i"):
            # tiled-contiguous layout: row block (dc*NT + gt)*P : +P is one
            # [128, 512] tile stored contiguously (single-descriptor DMA)
            x_ap[t + part] = nc.dram_tensor(
                f"x{t}_{part}", [DC * NT * P, TBLK],
                BF16, kind="ExternalInput").ap()
    # all projections use per-head combined weights: one psum directly
    # produces the attention layout ([q_r;q_i], [k_r;-k_i], [v_r;v_i])
    w_ap = {}
    for t in ("q", "k", "v"):
        for h in range(HPC):
            for suf in ("a", "b"):
                w_ap[f"{t}{suf}{h}"] = nc.dram_tensor(
                    f"w{t}_{suf}{h}", [P, D], BF16, kind="ExternalInput").ap()
    wo_ap = {}
    for suf in ("r", "i", "in"):
        wo_ap[suf] = nc.dram_tensor(
            f"wo_{suf}", [P, D], BF16, kind="ExternalInput").ap()
    ident_ap = nc.dram_tensor("ident", [P, P], BF16, kind="ExternalInput").ap()
    ones_ap = nc.dram_tensor("onesin", [P, P], BF16, kind="ExternalInput").ap()
    # same tiled-contiguous trick for outputs: row block (gt*DC + mc)*P
    po_r = nc.dram_tensor("po_r", [NT * DC * P, TBLK], BF16,
                          kind="ExternalOutput").ap()
    po_i = nc.dram_tensor("po_i", [NT * DC * P, TBLK], BF16,
                          kind="ExternalOutput").ap()

    with tile.TileContext(nc) as tc, ExitStack() as ctx:
        wpool = ctx.enter_context(tc.tile_pool(name="w", bufs=1))
        xpool = ctx.enter_context(tc.tile_pool(name="x", bufs=20))
        qkpool = ctx.enter_context(tc.tile_pool(name="qk", bufs=2))
        vpool = ctx.enter_context(tc.tile_pool(name="v", bufs=2))
        opool = ctx.enter_context(tc.tile_pool(name="ost", bufs=2))
        upool = ctx.enter_context(tc.tile_pool(name="u", bufs=8))
        uspool = ctx.enter_context(tc.tile_pool(name="us", bufs=8))
        zpool = ctx.enter_context(tc.tile_pool(name="z", bufs=2))
        tmppool = ctx.enter_context(tc.tile_pool(name="tmp", bufs=4))
        popool = ctx.enter_context(tc.tile_pool(name="po", bufs=4))
        vstpool = ctx.enter_context(tc.tile_pool(name="vst", bufs=2))
        # PSUM: 8 banks, split by consumer so engines don't cross-block.
        projps = ctx.enter_context(tc.tile_pool(name="pp", bufs=2, space="PSUM"))
        sps = ctx.enter_context(tc.tile_pool(name="sp", bufs=2, space="PSUM"))
        ojps = ctx.enter_context(tc.tile_pool(name="oj", bufs=2, space="PSUM"))
        ops_pool = ctx.enter_context(tc.tile_pool(name="op", bufs=1, space="PSUM"))

        wt = {}

        def load_w(key):
            wt[key] = wpool.tile([P, D], BF16, tag=f"w_{key}", name=f"w_{key}")
            nc.sync.dma_start(wt[key][:], w_ap[key][:])

        # per-batch staged tiles (created lazily, rotated by pool bufs=2)
        stage = {}

        def get_stage(b):
            if b not in stage:
                stage[b] = {
                    "qcat": [qkpool.tile([P, S], F32R, tag=f"qcat{h}",
                                         name=f"qcat{h}") for h in range(HPC)],
                    "kcr": [qkpool.tile([P, S], F32R, tag=f"kcr{h}",
                                        name=f"kcr{h}") for h in range(HPC)],
                    "kci": [qkpool.tile([P, S], F32R, tag=f"kci{h}",
                                        name=f"kci{h}") for h in range(HPC)],
                    "va": [vpool.tile([P, S], BF16, tag=f"va{h}",
                                      name=f"va{h}") for h in range(HPC)],
                    "vb": [vpool.tile([P, S], BF16, tag=f"vb{h}",
                                      name=f"vb{h}") for h in range(HPC)],
                    "o": {p: opool.tile([P, S], BF16, tag=f"ost{p}",
                                        name=f"ost{p}") for p in ("r", "i")},
                }
            return stage[b]

        def proj_unit_gen(b, t, half):
            """Projection of one (tensor, 512-token half): 32 matmuls,
            each DMA'd x tile feeding two of them. Yields after every
            chunk so the driver can interleave it into PE bubbles."""
            st = get_stage(b)
            gt = 2 * b + half
            wA = (wt[t + "a0"], wt[t + "a1"])
            wB = (wt[t + "b0"], wt[t + "b1"])
            psr = projps.tile([P, TBLK], F32, tag="projps", name="projps")
            psi = projps.tile([P, TBLK], F32, tag="projps", name="projps")
            for dc in range(DC):
                ws = slice(dc * P, (dc + 1) * P)
                r0 = (dc * NT + gt) * P
                xrt = xpool.tile([P, TBLK], BF16, tag="xt", name="xt")
                nc.sync.dma_start(xrt[:], x_ap[t + "r"][r0:r0 + P, :])
                nc.tensor.matmul(psr[:], wA[0][:, ws], xrt[:],
                                 start=(dc == 0), stop=False)
                nc.tensor.matmul(psi[:], wA[1][:, ws], xrt[:],
                                 start=(dc == 0), stop=False)
                yield
            for dc in range(DC):
                ws = slice(dc * P, (dc + 1) * P)
                r0 = (dc * NT + gt) * P
                xit = xpool.tile([P, TBLK], BF16, tag="xt", name="xt")
                nc.sync.dma_start(xit[:], x_ap[t + "i"][r0:r0 + P, :])
                nc.tensor.matmul(psr[:], wB[0][:, ws], xit[:],
                                 start=False, stop=(dc == DC - 1))
                nc.tensor.matmul(psi[:], wB[1][:, ws], xit[:],
                                 start=False, stop=(dc == DC - 1))
                yield
            hs = slice(half * TBLK, (half + 1) * TBLK)
            if t == "q":
                # psX = [q_r(h); q_i(h)] = Qcat directly
                for h, psx in ((0, psr), (1, psi)):
                    nc.vector.tensor_copy(st["qcat"][h][:, hs], psx[:])
            elif t == "k":
                # psX = [k_r(h); -k_i(h)] = Kcat_r directly;
                # Kcat_i = [k_i; k_r] via one negate + one copy
                for h, psx in ((0, psr), (1, psi)):
                    nc.vector.tensor_copy(st["kcr"][h][:, hs], psx[:])
                    nc.vector.tensor_scalar_mul(st["kci"][h][0:DH, hs],
                                                psx[DH:P, :], -1.0)
                    nc.vector.tensor_copy(st["kci"][h][DH:P, hs],
                                          psx[0:DH, :])
            else:
                # psr = [v_r(h0); v_i(h0)], psi = [v_r(h1); v_i(h1)]
                for h, psx in ((0, psr), (1, psi)):
                    vst = vstpool.tile([P, TBLK], BF16, tag="vst", name="vst")
                    nc.vector.tensor_copy(vst[:], psx[:])
                    ptb = projps.tile([P, TBLK], BF16, tag="projps",
                                      name="ptb")
                    for blk in range(4):
                        bs = slice(blk * P, (blk + 1) * P)
                        nc.tensor.transpose(ptb[:, bs], vst[:, bs], ident[:])
                    # ptb cols per blk: [v_r(h) 64 | v_i(h) 64]
                    base = half * TBLK
                    nc.vector.tensor_copy(st["va"][h][:, base:base + TBLK],
                                          ptb[:])
                    vbv = st["vb"][h][:, base:base + TBLK].rearrange(
                        "p (k c) -> p k c", c=P)
                    ptv = ptb[:].rearrange("p (k c) -> p k c", c=P)
                    nc.vector.tensor_copy(vbv[:, :, 0:DH], ptv[:, :, DH:P])
                    nc.vector.tensor_copy(vbv[:, :, DH:P], ptv[:, :, 0:DH])
                    yield

        # filler driver: background projection work pulled into PE bubbles
        fill_state = {"gen": None, "queue": deque()}

        def fill(n):
            for _ in range(n):
                while True:
                    if fill_state["gen"] is None:
                        if not fill_state["queue"]:
                            return
                        fill_state["gen"] = proj_unit_gen(
                            *fill_state["queue"].popleft())
                    try:
                        next(fill_state["gen"])
                        break
                    except StopIteration:
                        fill_state["gen"] = None

        def fill_drain():
            fill(1 << 30)

        def emit_attn_group(b, h, qb):
            """One (head, 512-query block): 32 score/AV matmuls + 2 Z,
            pulling filler work into every exp-gated bubble."""
            st = get_stage(b)
            qs = slice(qb * TBLK, (qb + 1) * TBLK)
            ota = ops_pool.tile([P, TBLK], F32, tag="ota", name="ota")
            otb = ops_pool.tile([P, TBLK], F32, tag="otb", name="otb")
            acc = {"r": [], "i": []}  # pairwise tree partials

            def tree_push(part, t_new):
                lst = acc[part]
                lst.append((0, t_new))
                while len(lst) >= 2 and lst[-1][0] == lst[-2][0]:
                    r1, a = lst.pop()
                    _, bt = lst.pop()
                    s = uspool.tile([P, TBLK], BF16, tag=f"us{part}",
                                    name=f"us{part}")
                    nc.vector.tensor_add(s[:], a[:], bt[:])
                    lst.append((r1 + 1, s))

            for kc in range(KC):
                ks = slice(kc * P, (kc + 1) * P)
                first, last = kc == 0, kc == KC - 1
                str_ = sps.tile([P, TBLK], F32, tag="sps", name="sps")
                nc.tensor.matmul(str_[:], st["kcr"][h][:, ks],
                                 st["qcat"][h][:, qs], start=True, stop=True)
                ur = upool.tile([P, TBLK], BF16, tag="u", name="u")
                nc.scalar.activation(ur[:], str_[:], EXP)
                sti = sps.tile([P, TBLK], F32, tag="sps", name="sps")
                nc.tensor.matmul(sti[:], st["kci"][h][:, ks],
                                 st["qcat"][h][:, qs], start=True, stop=True)
                ui = upool.tile([P, TBLK], BF16, tag="u", name="u")
                nc.scalar.activation(ui[:], sti[:], EXP)
                nc.tensor.matmul(ota[:], st["va"][h][:, ks], ur[:],
                                 start=first, stop=last)
                nc.tensor.matmul(otb[:], st["vb"][h][:, ks], ui[:],
                                 start=first, stop=last)
                tree_push("r", ur)
                tree_push("i", ui)
                fill(1)
            usum = {}
            for part in ("r", "i"):
                lst = acc[part]
                while len(lst) >= 2:  # KC is a power of 2, but be safe
                    _, a = lst.pop()
                    _, bt = lst.pop()
                    s = uspool.tile([P, TBLK], BF16, tag=f"us{part}",
                                    name=f"us{part}")
                    nc.vector.tensor_add(s[:], a[:], bt[:])
                    lst.append((99, s))
                usum[part] = lst[0][1]
            # Z replicated across partitions via one ones-matmul per part;
            # each AV term gets its OWN denominator (independent softmaxes).
            # Z psums come from the sps ring: no shared-bank serialization.
            zinv = {}
            for part in ("r", "i"):
                zps = sps.tile([P, TBLK], F32, tag="sps", name="zsum")
                nc.tensor.matmul(zps[:], ones[:], usum[part][:],
                                 start=True, stop=True)
                fill(1)
                zinv[part] = zpool.tile([P, TBLK], F32, tag="zinv",
                                        name=f"zinv{part}")
                nc.vector.reciprocal_approx_fast(zinv[part][:], zps[:])
            tmpa = tmppool.tile([P, TBLK], F32, tag="tmp", name="tmpa")
            nc.vector.tensor_mul(tmpa[:], ota[:], zinv["r"][:])
            tmpb = tmppool.tile([P, TBLK], F32, tag="tmp", name="tmpb")
            nc.vector.tensor_mul(tmpb[:], otb[:], zinv["i"][:])
            dst = slice(DH * h, DH * (h + 1))
            nc.vector.tensor_sub(st["o"]["r"][dst, qs], tmpa[0:DH, :],
                                 tmpb[0:DH, :])
            nc.vector.tensor_add(st["o"]["i"][dst, qs], tmpa[DH:P, :],
                                 tmpb[DH:P, :])
            fill(4)

        def emit_oproj(b, half):
            """Partial O-projection for one 512-token half: 32 matmuls
            in a dedicated psum ring drained by Scalar copies."""
            st = get_stage(b)
            hs = slice(half * TBLK, (half + 1) * TBLK)
            gt = 2 * b + half
            for mc in range(DC):
                ms = slice(mc * P, (mc + 1) * P)
                orow = (gt * DC + mc) * P
                pr = ojps.tile([P, TBLK], F32, tag="ojps", name="ojpr")
                nc.tensor.matmul(pr[:], wot["r"][:, ms], st["o"]["r"][:, hs],
                                 start=True, stop=False)
                nc.tensor.matmul(pr[:], wot["in"][:, ms], st["o"]["i"][:, hs],
                                 start=False, stop=True)
                sbr = popool.tile([P, TBLK], BF16, tag="po", name="po")
                nc.any.tensor_copy(sbr[:], pr[:])
                nc.sync.dma_start(po_r[orow:orow + P, :], sbr[:])
                pi = ojps.tile([P, TBLK], F32, tag="ojps", name="ojpi")
                nc.tensor.matmul(pi[:], wot["i"][:, ms], st["o"]["r"][:, hs],
                                 start=True, stop=False)
                nc.tensor.matmul(pi[:], wot["r"][:, ms], st["o"]["i"][:, hs],
                                 start=False, stop=True)
                sbi = popool.tile([P, TBLK], BF16, tag="po", name="po")
                nc.any.tensor_copy(sbi[:], pi[:])
                nc.sync.dma_start(po_i[orow:orow + P, :], sbi[:])
                fill(1)

        def run_unit(b, t, half):
            for _ in proj_unit_gen(b, t, half):
                pass

        # ---- prologue: weights in first-use order, batch-0 projections
        # start after only the q-weights are queued.
        for h in range(HPC):
            for suf in ("a", "b"):
                load_w(f"q{suf}{h}")
        ident = wpool.tile([P, P], BF16, tag="ident", name="ident")
        nc.sync.dma_start(ident[:], ident_ap[:])
        ones = wpool.tile([P, P], BF16, tag="ones", name="ones")
        nc.sync.dma_start(ones[:], ones_ap[:])
        run_unit(0, "q", 0)
        for h in range(HPC):
            for suf in ("a", "b"):
                load_w(f"k{suf}{h}")
        run_unit(0, "q", 1)
        run_unit(0, "k", 0)
        for h in range(HPC):
            for suf in ("a", "b"):
                load_w(f"v{suf}{h}")
        run_unit(0, "k", 1)
        run_unit(0, "v", 0)
        wot = {}
        for suf, ap in wo_ap.items():
            wot[suf] = wpool.tile([P, D], BF16, tag=f"wo_{suf}",
                                  name=f"wo_{suf}")
            nc.sync.dma_start(wot[suf][:], ap[:])
        run_unit(0, "v", 1)

        # ---- steady state: attention(b) with projection(b+1) pulled in
        # as fine-grained filler; oproj(b, half) as soon as its half of
        # o_stage completes. Unit order puts K/V (needed by the FIRST
        # group of b+1) ahead of q half 1 (needed only by the third).
        for b in range(B):
            if b + 1 < B:
                fill_state["queue"] = deque(
                    [(b + 1, "q", 0), (b + 1, "k", 0), (b + 1, "k", 1),
                     (b + 1, "v", 0), (b + 1, "v", 1), (b + 1, "q", 1)])
            emit_attn_group(b, 0, 0)
            emit_attn_group(b, 1, 0)
            emit_oproj(b, 0)
            emit_attn_group(b, 0, 1)
            emit_attn_group(b, 1, 1)
            emit_oproj(b, 1)
            fill_drain()
            stage.pop(b, None)

    nc.compile()
    return nc


def _w_sbuf_layout(w_t):
    """[D, 128] weight-transpose slice -> SBUF layout [128, dc*128+o]."""
    return np.ascontiguousarray(
        w_t.reshape(DC, P, P).transpose(1, 0, 2).reshape(P, D))


def _tile_x(xT, dtype):
    """[D, B*S] -> tiled-contiguous [DC*NT*P, TBLK] (rows: (dc*NT+gt)*P)."""
    t = xT.reshape(DC, P, NT, TBLK).transpose(0, 2, 1, 3)
    return np.ascontiguousarray(t.reshape(DC * NT * P, TBLK)).astype(dtype)


def _prepare_in_maps(inputs):
    bf = ml_dtypes.bfloat16
    xs = {}
    for name, t in (("queries", "q"), ("keys", "k"), ("values", "v")):
        x = np.asarray(inputs[name], dtype=np.float32)  # [B,S,D,2]
        flat = x.reshape(B * S, D, 2)
        xs[t + "r"] = _tile_x(flat[:, :, 0].T, bf)
        xs[t + "i"] = _tile_x(flat[:, :, 1].T, bf)

    scale = np.float32(1.0 / np.sqrt(DH))
    in_maps = []
    for c in range(NCORES):
        rows = slice(P * c, P * (c + 1))
        m = {}
        for t in ("q", "k", "v"):
            for part in ("r", "i"):
                m[f"x{t}_{part}"] = xs[t + part]
        for t, wr_name, wi_name in (("q", "wq_r", "wq_i"),
                                    ("k", "wk_r", "wk_i"),
                                    ("v", "wv_r", "wv_i")):
            s = scale if t == "q" else np.float32(1.0)
            wr = np.asarray(inputs[wr_name], dtype=np.float32)[rows] * s
            wi = np.asarray(inputs[wi_name], dtype=np.float32)[rows] * s
            for h in range(HPC):
                hr = slice(DH * h, DH * (h + 1))
                if t == "q":
                    wa = np.concatenate([wr[hr].T, wi[hr].T], axis=1)
                    wb = np.concatenate([-wi[hr].T, wr[hr].T], axis=1)
                elif t == "k":
                    wa = np.concatenate([wr[hr].T, -wi[hr].T], axis=1)
                    wb = np.concatenate([-wi[hr].T, -wr[hr].T], axis=1)
                else:
                    wa = np.concatenate([wr[hr].T, wi[hr].T], axis=1)
                    wb = np.concatenate([-wi[hr].T, wr[hr].T], axis=1)
                m[f"w{t}_a{h}"] = _w_sbuf_layout(wa).astype(bf)
                m[f"w{t}_b{h}"] = _w_sbuf_layout(wb).astype(bf)
        wo_r = np.asarray(inputs["wo_r"], dtype=np.float32)[:, rows]  # [D,128]
        wo_i = np.asarray(inputs["wo_i"], dtype=np.float32)[:, rows]
        m["wo_r"] = np.ascontiguousarray(wo_r.T).astype(bf)  # [128 d, 1024 m]
        m["wo_i"] = np.ascontiguousarray(wo_i.T).astype(bf)
        m["wo_in"] = np.ascontiguousarray(-wo_i.T).astype(bf)
        m["ident"] = np.eye(P, dtype=bf)
        m["onesin"] = np.ones((P, P), dtype=bf)
        in_maps.append(m)
    return in_maps


LAST_RESULT = None


def _run(inputs, trace=False):
    global LAST_RESULT
    from concourse.bass_utils import run_bass_kernel_spmd
    if "nc" not in _CACHE:
        _CACHE["nc"] = _build()
    nc = _CACHE["nc"]
    in_maps = _prepare_in_maps(inputs)
    if trace:
        os.environ.pop("BASS_NEVER_TRACE", None)
    else:
        os.environ["BASS_NEVER_TRACE"] = "1"
    res = run_bass_kernel_spmd(nc, in_maps, core_ids=list(range(NCORES)),
                               trace=trace)
    LAST_RESULT = res
    acc_r = np.zeros((NT * DC * P, TBLK), np.float32)
    acc_i = np.zeros((NT * DC * P, TBLK), np.float32)
    for c in range(NCORES):
        acc_r += res.results[c]["po_r"].astype(np.float32)
        acc_i += res.results[c]["po_i"].astype(np.float32)

    def untile(po):
        # [NT*DC*P, TBLK] rows (gt*DC+mc)*P -> [D, B*S] -> [B,S,D]
        t = po.reshape(NT, DC, P, TBLK).transpose(1, 2, 0, 3)
        return np.ascontiguousarray(t.reshape(D, B * S)).T.reshape(B, S, D)

    out = np.empty((B, S, D, 2), np.float32)
    out[..., 0] = untile(acc_r)
    out[..., 1] = untile(acc_i)
    return out


def kernel(**inputs):
    return _run(inputs, trace=False)
